# revision 45
# baseline (speedup 1.0000x reference)
"""NTXEnt (intra-sample) loss kernel for Trainium2, 8-core data-parallel.

Math (matches the jax reference):
  inp [C=8, V=2, B=4096, D=512] fp32
  xn = inp / max(||inp||_D, 1e-12)
  sim[i,b,jv] = <xn[i,0,b], xn[jv,b]> / T          (T = 0.1)
  loss[i,b]   = log( sum_{jv != (i,0)} exp(sim[i,b,jv]) ) - sim[i,b,(i,1)]
  answer = mean over (i, b).

Per-core plan (b_loc = 512 rows, 4 chunks of 128), all 4 chunks software-
pipelined (casts prefetched first, then all compute prefixes, then all
PSUM drains):
  1. SWDGE cast-DMA: HBM fp32 -> SBUF bf16 [128 b, 16 iv, 512 d] (the
     dtype cast rides the DMA; no engine time).
  2. norms^2 via per-iv square+accumulate, split ScalarE (Square) / DVE
     (STT x*x) / GPSIMD (tensor_tensor square + 4x DVE reduce; Pool has
     no TensorScalarPtr opcode).
  3. r = 1/max(||x||,eps) = exp(-0.5 ln(max(nn,eps^2))); Square/Ln/Exp all
     live in one ACT table set (natural_log_exp_and_others, pinned via
     _patch_act_tables so the scheduler cannot thrash table loads).
  4. normalize: 16x DVE tensor_scalar_mul with per-partition scalar
     r[:,iv] (bf16 4x mode).
  5. one xbar transpose-DMA per chunk: [128 b, 8192] -> xnT [128 dsub,
     (iv,k), 128 b]; plus a 4x bf16 DVE gather of the 64 anchor (even-iv)
     columns per (k, g) into contiguous xnS (walrus requires the matmul
     stationary AP to be single-free-dim).
  6. PE grams, col-tiled: region r packs b-groups 2r / 2r+1 at
     tile_position (0,0) / (0,64); stationary = 64 anchors, moving =
     [128 d, 128 (iv,b)]; 4 K-chunks accumulate in PSUM. 2 banks/chunk.
  7. DVE STT from PSUM: W = (T*10) .* BD01 (static block-diag mask, self
     column zeroed). Masked entries are exactly 0 -> exp contributes 1.0
     each; there are 113 of them per row.
  8. ScalarE exp(W) -> E (bf16); per-region D row-sums via 4x DVE
     tensor_scalar accumulate; lnD = Ln(D - 113) via bias AP.
     pos logits: DVE STT (T*10) .* MPOS with accumulate.
  9. host: loss = (sum lnD - sum pos) / (C*B).
"""

import os
import numpy as np

C, V, B, D = 8, 2, 4096, 512
NCORES = 8
B_LOC = B // NCORES            # 512
P = 128                        # partitions per chunk
NCH = B_LOC // P               # 4 chunks
IV = C * V                     # 16
KCH = D // P                   # 4 contraction chunks
NG = P // C                    # 16 b-groups of 8 per chunk
SCALE = 10.0                   # 1 / temperature
EPS2 = 1e-24                   # (1e-12)^2 clamp on ||x||^2
CORR = float(P - 15)           # masked columns contributing exp(0)=1

# norms engine split per chunk: (scalar, dve, gpsimd) counts summing IV
_SPLIT = tuple(int(x) for x in
               os.environ.get("NTX_NORM_SPLIT", "8,6,2").split(","))


def _patch_act_tables():
    """Constrain square/ln/exp to the one table set containing all three
    (natural_log_exp_and_others) so the act-table-load pass emits a single
    load instead of thrashing between per-function default sets.  Set ids
    stay valid: the dict order is unchanged, only membership is filtered."""
    import functools
    import concourse.bacc as bacc_mod
    import concourse.hw_specs as hw_specs
    import concourse.mybir as mybir

    orig = hw_specs.get_activation_tables
    pinned = {mybir.ActivationFunctionType.Square,
              mybir.ActivationFunctionType.Ln,
              mybir.ActivationFunctionType.Exp}

    @functools.cache
    def patched(arch):
        tables = dict(orig(arch))
        out = {}
        for name, funcs in tables.items():
            if name == "natural_log_exp_and_others":
                out[name] = set(funcs)
            else:
                out[name] = set(funcs) - pinned
        return out

    bacc_mod.get_activation_tables = patched


def _masks():
    """Masks for the col-tiled layout: PSUM rows m = (half, i, b) with
    half in {0,1} selecting the b-group, i = anchor crop (0..7), b = m % 8;
    cols n = (iv, b') with iv = n // 8, b' = n % 8 (within that half's
    b-group).  Keep col iff b' == b and iv != 2i; pos col is (2i+1, b)."""
    m = np.arange(P)
    n = np.arange(P)
    im, bm = (m % 64) // C, m % C
    ivn, bn = n // C, n % C
    same_b = bn[None, :] == bm[:, None]
    selfv0 = ivn[None, :] == (2 * im)[:, None]
    bd01 = (same_b & ~selfv0).astype(np.float32)
    mpos = np.zeros((P, P), np.float32)
    mpos[m, (2 * im + 1) * C + bm] = 1.0
    return bd01, mpos


def _build_kernel(norm_split=_SPLIT):
    from contextlib import ExitStack

    import concourse.bacc as bacc
    import concourse.tile as tile
    import concourse.mybir as mybir

    _patch_act_tables()

    f32 = mybir.dt.float32
    bf16 = mybir.dt.bfloat16
    Alu = mybir.AluOpType
    Act = mybir.ActivationFunctionType

    n_sc, n_dv, n_gp = norm_split
    assert n_sc + n_dv + n_gp == IV

    nc = bacc.Bacc("TRN2", target_bir_lowering=False, debug=False)
    x_d = nc.dram_tensor("inp", [C, V, B_LOC, D], f32, kind="ExternalInput")
    bd_d = nc.dram_tensor("bd01", [P, P], f32, kind="ExternalInput")
    mp_d = nc.dram_tensor("mpos", [P, P], f32, kind="ExternalInput")
    # out: lnD [128, NCH*8 regions] then pos partial sums [128, NCH*2]
    o_d = nc.dram_tensor("out", [P, NCH * 10], f32, kind="ExternalOutput")

    with tile.TileContext(nc) as tc, ExitStack() as ctx:
        const = ctx.enter_context(tc.tile_pool(name="const", bufs=1))
        xp = ctx.enter_context(tc.tile_pool(name="x", bufs=NCH))
        np_ = ctx.enter_context(tc.tile_pool(name="xn", bufs=2))
        tp = ctx.enter_context(tc.tile_pool(name="xt", bufs=3))
        wp = ctx.enter_context(tc.tile_pool(name="w", bufs=2))
        scr = ctx.enter_context(tc.tile_pool(name="scr", bufs=2))
        small = ctx.enter_context(tc.tile_pool(name="small", bufs=2))
        outp = ctx.enter_context(tc.tile_pool(name="outp", bufs=1))
        psp = ctx.enter_context(tc.psum_pool(name="ps", bufs=NCH))

        bd01 = const.tile([P, P], f32)
        mpos = const.tile([P, P], f32)
        bias = const.tile([P, 1], f32)
        nc.vector.memset(bias[:, :], -CORR)

        obuf = outp.tile([P, NCH * 10], f32)
        x_ap = x_d.ap()

        def issue_cast(c):
            """Prefetch chunk c's cast-load; issued ahead so the Pool queue
            never parks a descriptor-gen behind norms of the prior chunk."""
            xbf = xp.tile([P, IV, D], bf16, tag="xbf", name="xbf")
            src = x_ap[:, :, c * P:(c + 1) * P, :].rearrange(
                "i v b d -> b (i v) d")
            with tc.high_priority():
                nc.gpsimd.dma_start(out=xbf[:, :, :], in_=src)
            return xbf

        def front(c, xbf):
            """norms -> r -> normalize -> transpose -> PE grams.
            Returns the chunk's PSUM bank tiles."""
            nn = small.tile([P, IV], f32, tag="nn", name="nn")
            for iv in range(IV):
                if iv < n_sc:
                    sq = scr.tile([P, D], bf16, tag="sq_s", name="sq")
                    nc.scalar.activation(
                        out=sq[:, :], in_=xbf[:, iv, :], func=Act.Square,
                        accum_out=nn[:, iv:iv + 1])
                elif iv < n_sc + n_dv:
                    sq = scr.tile([P, D], bf16, tag="sq_v", name="sq")
                    nc.vector.scalar_tensor_tensor(
                        out=sq[:, :], in0=xbf[:, iv, :], scalar=1.0,
                        in1=xbf[:, iv, :], op0=Alu.mult, op1=Alu.mult,
                        accum_out=nn[:, iv:iv + 1])
                else:
                    # Pool has no TensorScalarPtr opcode: square on GPSIMD
                    # via tensor_tensor, reduce with a 4x DVE tensor_scalar.
                    sq = scr.tile([P, D], bf16, tag="sq_g", name="sq")
                    nc.gpsimd.tensor_tensor(
                        out=sq[:, :], in0=xbf[:, iv, :], in1=xbf[:, iv, :],
                        op=Alu.mult)
                    sq2 = scr.tile([P, D], bf16, tag="sq_g2", name="sq2")
                    nc.vector.tensor_scalar(
                        out=sq2[:, :], in0=sq[:, :], scalar1=0.0,
                        scalar2=None, op0=Alu.add, op1=Alu.add,
                        accum_out=nn[:, iv:iv + 1])

            nnc = small.tile([P, IV], f32, tag="nnc", name="nnc")
            nc.vector.tensor_scalar_max(nnc[:, :], nn[:, :], EPS2)
            lnn = small.tile([P, IV], f32, tag="lnn", name="lnn")
            nc.scalar.activation(out=lnn[:, :], in_=nnc[:, :], func=Act.Ln)
            r = small.tile([P, IV], f32, tag="r", name="r")
            nc.scalar.activation(out=r[:, :], in_=lnn[:, :], func=Act.Exp,
                                 scale=-0.5)

            xn = np_.tile([P, IV, D], bf16, tag="xn", name="xn")
            for iv in range(IV):
                nc.vector.tensor_scalar_mul(
                    xn[:, iv, :], xbf[:, iv, :], r[:, iv:iv + 1])

            # The last chunk's transpose is split into two b-halves: its PE
            # and drain work sit on the critical-path tail, and regions 0-3
            # only need b rows 0-63, so compute starts after half a
            # transpose (costs one extra DMA fixed overhead).
            nh = 2 if c == NCH - 1 else 1
            bw = P // nh
            xnT = tp.tile([P, IV, KCH, P], bf16, tag="xnT", name="xnT")
            xnS = tp.tile([P, KCH, NG, C, C], bf16, tag="xnS", name="xnS")
            src = xnT[:, 0:IV:2, :, :].rearrange("p i k (g b) -> p k g i b",
                                                 g=NG)
            pg = [psp.tile([P, 4, P], f32, tag=f"pg{q}", name=f"pg{q}")
                  for q in range(2)]
            for h in range(nh):
                nc.sync.dma_start_transpose(
                    out=xnT[:, :, :, h * bw:(h + 1) * bw].rearrange(
                        "p i k b -> p (i k) b"),
                    in_=xn[h * bw:(h + 1) * bw, :, :].rearrange(
                        "b i d -> b (i d)"))
                # stationary operands must be single-free-dim APs (walrus
                # LHS constraint) - gather the anchor (even-iv) columns per
                # (k, g) into a contiguous tile with 4x bf16 DVE copies.
                g0, g1 = h * NG // nh, (h + 1) * NG // nh
                with tc.high_priority():
                    for k in range(KCH):
                        nc.vector.tensor_copy(
                            out=xnS[:, k, g0:g1, :, :], in_=src[:, k, g0:g1])

                # col-tiled grams: region r packs b-groups 2r (psum rows
                # 0-63) and 2r+1 (rows 64-127); stationary = 64 anchors,
                # moving = all 128 columns of that b-group.
                for r in range(h * 8 // nh, (h + 1) * 8 // nh):
                    for half, cp in ((0, 0), (1, 64)):
                        g = 2 * r + half
                        pt = pg[r // 4][cp:cp + 64, r % 4, :]
                        for k in range(KCH):
                            st = xnS[:, k, g, :, :]
                            mv = xnT[:, :, k, C * g:C * (g + 1)]
                            nc.tensor.matmul(pt, st, mv, start=(k == 0),
                                             stop=(k == KCH - 1),
                                             tile_position=(0, cp))
            return pg

        def back(c, pg):
            """mask, exp, D-sums, pos, lnD for chunk c's PSUM tiles."""
            draw = small.tile([P, 8], f32, tag="draw", name="draw")
            for q in range(2):
                W = wp.tile([P, 4, P], f32, tag="W", name="W")
                bd_b = bd01[:, :].unsqueeze(1).broadcast_to([P, 4, P])
                nc.vector.scalar_tensor_tensor(
                    out=W[:, :, :], in0=pg[q][:, :, :], scalar=SCALE,
                    in1=bd_b, op0=Alu.mult, op1=Alu.mult)
                Wp = wp.tile([P, 4, P], f32, tag="Wp", name="Wp")
                mp_b = mpos[:, :].unsqueeze(1).broadcast_to([P, 4, P])
                nc.vector.scalar_tensor_tensor(
                    out=Wp[:, :, :], in0=pg[q][:, :, :], scalar=SCALE,
                    in1=mp_b, op0=Alu.mult, op1=Alu.mult,
                    accum_out=obuf[:, NCH * 8 + c * 2 + q:
                                   NCH * 8 + c * 2 + q + 1])
                E = scr.tile([P, 4, P], bf16, tag="E", name="E")
                nc.scalar.activation(out=E[:, :, :], in_=W[:, :, :],
                                     func=Act.Exp)
                for t in range(4):
                    ts_scr = scr.tile([P, P], bf16, tag="ts", name="ts")
                    nc.vector.tensor_scalar(
                        out=ts_scr[:, :], in0=E[:, t, :], scalar1=0.0,
                        scalar2=None, op0=Alu.add, op1=Alu.add,
                        accum_out=draw[:, 4 * q + t:4 * q + t + 1])

            nc.scalar.activation(
                out=obuf[:, c * 8:(c + 1) * 8], in_=draw[:, :],
                func=Act.Ln, bias=bias[:, 0:1])

        # software-pipelined: all cast-loads first, then all compute
        # prefixes (so every transpose can issue as soon as its normalize
        # lands), then all PSUM drains.  PSUM pool bufs=2 inserts the
        # bank-reuse deps between front(c+2) and back(c) automatically.
        xbfs = [issue_cast(c) for c in range(NCH)]
        # mask loads ride the DMA queue behind the cast prefetches; they are
        # not needed until the first PSUM drain (~35 us in)
        nc.sync.dma_start(out=bd01[:, :], in_=bd_d.ap())
        nc.sync.dma_start(out=mpos[:, :], in_=mp_d.ap())
        pgs = [front(c, xbfs[c]) for c in range(NCH)]
        for c in range(NCH):
            back(c, pgs[c])

        nc.sync.dma_start(out=o_d.ap(), in_=obuf[:, :])

    nc.compile()
    return nc


_CACHE = {}


def _get_nc(norm_split=_SPLIT):
    key = norm_split
    if key not in _CACHE:
        _CACHE[key] = _build_kernel(norm_split)
    return _CACHE[key]


def _host_reduce(results):
    total = np.float64(0.0)
    for m in results:
        o = m["out"].astype(np.float64)
        lnD = o[:, :NCH * 8]
        pos = o[:, NCH * 8:]
        total += lnD.sum() - pos.sum()
    return np.float32(total / (C * B))


def _run(inp, trace=False):
    from concourse.bass_utils import run_bass_kernel_spmd

    nc = _get_nc()
    bd01, mpos = _masks()
    in_maps = []
    for k in range(NCORES):
        shard = np.ascontiguousarray(inp[:, :, k * B_LOC:(k + 1) * B_LOC, :],
                                     dtype=np.float32)
        in_maps.append({"inp": shard, "bd01": bd01, "mpos": mpos})
    res = run_bass_kernel_spmd(nc, in_maps, list(range(NCORES)), trace=trace)
    return _host_reduce(res.results), res


def kernel(inp):
    loss, _ = _run(np.asarray(inp), trace=False)
    return loss
